# revision 25
# baseline (speedup 1.0000x reference)
"""Trainium2 Bass kernel for nn_CausalGCN (RGCN + GAT message passing).

Sharding: data-parallel over graphs. Each of the 8 cores takes 32 of the
256 dialogs; the per-graph block-diagonal edge structure means no
cross-core edges. Small RGCN/GAT weights are replicated to every core.

Per-core math (N_loc = 320 nodes = 32 tgt + 32 emo + 256 cause, D = 600):
  RGCN   out1 = sum_r (Cn_r @ x) @ W_r + x @ root + b
         with Cn_r the row-normalized per-relation adjacency (dense
         [320,320], built on host from edge_index as part of sharding;
         mean-then-project == project-then-mean since W_r is linear).
  GAT    h = out1 @ gat_w; dense masked softmax over incoming edges
         (+self loops); out2 = coef @ h + b.
  Output out_final = broadcast of cause rows of out2 over S=128, written
         directly from SBUF with stride-0 source DMAs (memory-bound part).

All matmuls run fp32 on the PE (fp32 streams at the same N-cycles rate as
bf16 on trn2, so compute stays far below the HBM-write roofline).
"""

import sys

for _p in ("/opt/trn_rl_repo", "/root/.axon_site/_ro/trn_rl_repo"):
    if _p not in sys.path:
        sys.path.insert(0, _p)

import numpy as np

import concourse.bacc as bacc
import concourse.mybir as mybir
import concourse.tile as tile
from concourse.bass_utils import run_bass_kernel_spmd

# Problem constants (hardcoded per the task contract).
B = 256       # graphs (dialogs)
K = 8         # cause utterances per graph
S = 128       # sequence length of the broadcast output
D = 600       # node/hidden dim
R = 8         # relations
NCORES = 8
G = B // NCORES          # 32 graphs per core
NL = G * (K + 2)         # 320 local nodes per core
NC_CAUSE = G * K         # 256 local cause nodes per core
KDIM = R * D + D         # 5400 contraction dim for [Wr..., root]

FP32 = mybir.dt.float32
FP32R = mybir.dt.float32r  # fp32 bits, 4x faster PE streaming at N>=256

# Node-chunk tiling along the 320-node axis (PE partition limit 128).
N_CHUNKS = [(0, 128), (128, 128), (256, 64)]
# Feature-dim slabs along D=600 (5 x 120).
D_SLABS = [(j * 120, 120) for j in range(5)]

_compiled = None  # (nc, ) cache — compile once per process


def _build_program(repeat=1):
    nc = bacc.Bacc("TRN2", target_bir_lowering=False, debug=False,
                   num_devices=NCORES)

    x_d = nc.dram_tensor("x", [3 * 128, D], FP32R, kind="ExternalInput")
    xt_d = nc.dram_tensor("xt", [D, NL], FP32R, kind="ExternalInput")
    # ct padded to k-slabs of 128 (zero rows) so it loads as one DMA
    ct_d = nc.dram_tensor("ct", [R * 3 * 128, NL], FP32R,
                          kind="ExternalInput")
    mb_d = nc.dram_tensor("mb", [NL, NL], FP32, kind="ExternalInput")
    wcat_d = nc.dram_tensor("wcat", [KDIM, D], FP32R, kind="ExternalInput")
    gw_d = nc.dram_tensor("gw", [D, D], FP32R, kind="ExternalInput")
    att_d = nc.dram_tensor("att", [D, 2], FP32R, kind="ExternalInput")
    rb_d = nc.dram_tensor("rb", [120, 5], FP32, kind="ExternalInput")
    id_d = nc.dram_tensor("idf", [128, 128], FP32, kind="ExternalInput")
    idr_d = nc.dram_tensor("idr", [128, 128], FP32R, kind="ExternalInput")
    ones_d = nc.dram_tensor("ones", [1, 128], FP32R, kind="ExternalInput")
    gb_d = nc.dram_tensor("gb", [1, D], FP32, kind="ExternalInput")

    out1_d = nc.dram_tensor("out1", [NL, D], FP32, kind="ExternalOutput")
    out2_d = nc.dram_tensor("out2", [NL, D], FP32, kind="ExternalOutput")
    outf_d = nc.dram_tensor("outf", [NC_CAUSE, S, D], FP32,
                            kind="ExternalOutput")

    dd = dict(x_d=x_d, xt_d=xt_d, ct_d=ct_d, mb_d=mb_d, wcat_d=wcat_d,
              gw_d=gw_d, att_d=att_d, rb_d=rb_d, id_d=id_d, idr_d=idr_d,
              ones_d=ones_d, gb_d=gb_d, out1_d=out1_d, out2_d=out2_d,
              outf_d=outf_d)
    with tile.TileContext(nc) as tc:
        if repeat == 1:
            _emit_body(nc, tc, dd)
        else:
            with tc.For_i(0, repeat, 1):
                _emit_body(nc, tc, dd)

    nc.compile()
    return nc


def _emit_body(nc, tc, dd):
    (x_d, xt_d, ct_d, mb_d, wcat_d, gw_d, att_d, rb_d, id_d, idr_d,
     ones_d, gb_d, out1_d, out2_d, outf_d) = (
        dd[k] for k in ("x_d", "xt_d", "ct_d", "mb_d", "wcat_d", "gw_d",
                        "att_d", "rb_d", "id_d", "idr_d", "ones_d",
                        "gb_d", "out1_d", "out2_d", "outf_d"))

    def mmr(out, lhsT, rhs, start, stop):
        nc.tensor.matmul(out, lhsT, rhs, start=start, stop=stop)

    if True:
        with (
            tc.tile_pool(name="const", bufs=1) as const,
            tc.tile_pool(name="ctp", bufs=1) as ctp,
            tc.tile_pool(name="aggp", bufs=1) as aggp,
            tc.tile_pool(name="wstream", bufs=2) as wstream,
            tc.tile_pool(name="o1t", bufs=1) as o1tp,
            tc.tile_pool(name="hp", bufs=1) as hp,
            tc.tile_pool(name="htp", bufs=1) as htp,
            tc.tile_pool(name="smax", bufs=1) as smax,
            tc.tile_pool(name="outp", bufs=1) as outp,
        ):
            # ---- resident inputs -------------------------------------
            # x loaded into zero-padded k-slab layout [128, 3, D]
            xpad = ctp.tile([128, 3, D], FP32R, tag="xpad")
            nc.sync.dma_start(
                xpad[:], x_d[:].rearrange("(k p) d -> p k d", k=3))
            xt_sb = []
            for (j0, jj) in D_SLABS:
                t = const.tile([jj, NL], FP32R, tag=f"xt{j0}")
                nc.sync.dma_start(t[:], xt_d[j0:j0 + jj, :])
                xt_sb.append(t)
            gw_all = const.tile([120, 5, D], FP32R, tag="gw")
            nc.sync.dma_start(
                gw_all[:], gw_d[:].rearrange("(s p) d -> p s d", s=5))
            gw_sb = [gw_all[:, j, :] for j in range(5)]
            att_sb = []
            for (j0, jj) in D_SLABS:
                t = const.tile([jj, 2], FP32R, tag=f"att{j0}")
                nc.sync.dma_start(t[:], att_d[j0:j0 + jj, :])
                att_sb.append(t)
            rb_sb = const.tile([120, 5], FP32, tag="rb")
            nc.sync.dma_start(rb_sb[:], rb_d[:])
            gb_bc = const.tile([128, D], FP32, tag="gb")
            nc.sync.dma_start(gb_bc[:], gb_d[0:1, :].broadcast_to([128, D]))
            mb_sb = []
            for (n0, nn) in N_CHUNKS:
                t = const.tile([nn, NL], FP32, tag=f"mb{n0}")
                nc.sync.dma_start(t[:], mb_d[n0:n0 + nn, :])
                mb_sb.append(t)
            ident = const.tile([128, 128], FP32, tag="ident")
            identr = const.tile([128, 128], FP32R, tag="identr")
            nc.sync.dma_start(ident[:], id_d[:])
            nc.sync.dma_start(identr[:], idr_d[:])

            # ---- stage A: aggT_r[di, n] = sum_s x[s, di] * Cn_r[s, n] --
            # ct_sb[:, r*3+kk, :] is the (r, k-slab) rhs; last slab is
            # zero-padded from 64 to 128 rows so it loads as one DMA.
            ct_sb = ctp.tile([128, R * 3, NL], FP32R, tag="ct")
            nc.sync.dma_start(
                ct_sb[:], ct_d[:].rearrange("(rk p) d -> p rk d", rk=R * 3))
            agg_sb = []
            with tc.tile_pool(name="psA", bufs=2, space="PSUM") as psA:
                for r in range(R):
                    agg_r = aggp.tile([120, 5, NL], FP32R, tag=f"agg{r}")
                    for mj, (m0, mw) in enumerate(D_SLABS):
                        ps = psA.tile([120, NL], FP32, tag="psA")
                        for kk in range(3):
                            mmr(ps[:], xpad[:, kk, m0:m0 + mw],
                                ct_sb[:, r * 3 + kk, :],
                                start=(kk == 0), stop=(kk == 2))
                        nc.vector.tensor_copy(agg_r[:, mj, :], ps[:])
                    agg_sb.append(agg_r)

                # ---- stage B: out1T = wcat.T-contract, k-streamed ------
                with tc.tile_pool(name="psB", bufs=1, space="PSUM") as psB:
                    psb = [psB.tile([120, NL], FP32, tag=f"psB{m}",
                                    name=f"psb{m}")
                           for m in range(5)]
                    nk = R * 5 + 5
                    for k9 in range(9):  # 9 super-slabs of 5 k-slabs each
                        wt = wstream.tile([120, 5, D], FP32R, tag="w")
                        nc.sync.dma_start(
                            wt[:],
                            wcat_d[k9 * 600:(k9 + 1) * 600, :].rearrange(
                                "(s p) d -> p s d", s=5))
                        for s in range(5):
                            k = k9 * 5 + s
                            if k < R * 5:
                                rhs = agg_sb[k // 5][:, k % 5, :]
                            else:
                                rhs = xt_sb[k - R * 5][:]
                            for mj, (m0, mw) in enumerate(D_SLABS):
                                mmr(psb[mj][:], wt[:, s, m0:m0 + mw], rhs,
                                    start=(k == 0), stop=(k == nk - 1))
                    out1t_sb = []
                    out1tr_sb = []
                    for mj in range(5):
                        t = o1tp.tile([120, NL], FP32, tag=f"o1t{mj}")
                        nc.vector.tensor_scalar_add(
                            t[:], psb[mj][:], rb_sb[:, mj:mj + 1])
                        out1t_sb.append(t)
                        tr = o1tp.tile([120, NL], FP32R, tag=f"o1tr{mj}",
                                       name=f"o1tr{mj}")
                        nc.vector.tensor_scalar_add(
                            tr[:], psb[mj][:], rb_sb[:, mj:mj + 1])
                        out1tr_sb.append(tr)

            # ---- stage C: h (native), hT, attention alphas -----------
            h_sb = []
            with (
                tc.tile_pool(name="psC", bufs=2, space="PSUM") as psC,
                tc.tile_pool(name="psD", bufs=2, space="PSUM") as psD,
                tc.tile_pool(name="psE", bufs=1, space="PSUM") as psE,
            ):
                for ti, (n0, nn) in enumerate(N_CHUNKS):
                    t = hp.tile([nn, D], FP32R, tag=f"h{ti}")
                    for half in range(2):
                        ps = psC.tile([nn, 300], FP32, tag="psC")
                        for j in range(5):
                            mmr(ps[:], out1tr_sb[j][:, n0:n0 + nn],
                                gw_sb[j][:, half * 300:(half + 1) * 300],
                                start=(j == 0), stop=(j == 4))
                        nc.vector.tensor_copy(
                            t[:, half * 300:(half + 1) * 300], ps[:])
                    h_sb.append(t)
                ht_sb = []
                for j2, (o0, oo) in enumerate(D_SLABS):
                    t = htp.tile([oo, NL], FP32R, tag=f"ht{j2}")
                    ps = psD.tile([oo, NL], FP32, tag="psD")
                    for j in range(5):
                        mmr(ps[:], gw_sb[j][:, o0:o0 + oo], out1tr_sb[j][:],
                            start=(j == 0), stop=(j == 4))
                    nc.vector.tensor_copy(t[:], ps[:])
                    ht_sb.append(t)
                psal = psE.tile([2, NL], FP32, tag="psE")
                for j in range(5):
                    mmr(psal[:], att_sb[j][:], ht_sb[j][:],
                        start=(j == 0), stop=(j == 4))
                alphas = smax.tile([2, NL], FP32R, tag="alphas")
                nc.vector.tensor_copy(alphas[:], psal[:])

                # a_src broadcast row: ones[1,128].T @ alphas[0:1,:]
                ones_row = smax.tile([1, 128], FP32R, tag="ones")
                nc.sync.dma_start(ones_row[:], ones_d[:])
                asrc_bc = smax.tile([128, NL], FP32, tag="asrc")
                psbc = psE.tile([128, NL], FP32, tag="psBc")
                mmr(psbc[:], ones_row[:], alphas[0:1, :],
                    start=True, stop=True)
                nc.vector.tensor_copy(asrc_bc[:], psbc[:])
            acol_sb = []
            with tc.tile_pool(name="psT", bufs=2, space="PSUM") as psT:
                for ti, (n0, nn) in enumerate(N_CHUNKS):
                    ps = psT.tile([nn, 2], FP32R, tag="psTa")
                    nc.tensor.transpose(ps[:], alphas[:, n0:n0 + nn],
                                        identr[0:2, 0:2])
                    t = smax.tile([nn, 2], FP32, tag=f"acol{ti}")
                    nc.vector.tensor_copy(t[:], ps[:])
                    acol_sb.append(t)

                # ---- stage D: masked softmax over incoming edges ------
                coef_sb = []
                for ti, (n0, nn) in enumerate(N_CHUNKS):
                    L = smax.tile([nn, NL], FP32, tag=f"L{ti}")
                    nc.vector.tensor_scalar_add(
                        L[:], asrc_bc[0:nn, :], acol_sb[ti][:, 1:2])
                    # leaky_relu(x, 0.2) = max(x, 0.2x)
                    Lm = smax.tile([nn, NL], FP32, tag="Lm")
                    nc.vector.tensor_scalar_mul(Lm[:], L[:], 0.2)
                    nc.vector.tensor_max(L[:], L[:], Lm[:])
                    nc.vector.tensor_add(L[:], L[:], mb_sb[ti][:])
                    namax = smax.tile([nn, 1], FP32, tag="namax")
                    nc.vector.tensor_reduce(
                        namax[:], L[:], axis=mybir.AxisListType.X,
                        op=mybir.AluOpType.max, negate=True)
                    P = smax.tile([nn, NL], FP32R, tag=f"P{ti}")
                    den = smax.tile([nn, 1], FP32, tag="den")
                    nc.scalar.activation(
                        P[:], L[:], mybir.ActivationFunctionType.Exp,
                        bias=namax[:], accum_out=den[:])
                    rden = smax.tile([nn, 1], FP32, tag="rden")
                    nc.vector.reciprocal(rden[:], den[:])
                    nc.vector.tensor_scalar_mul(P[:], P[:], rden[:])
                    coef_sb.append(P)

                # ---- stage E: coefT via PE transpose ------------------
                coeft_sb = []
                for ui, (u0, uu) in enumerate(N_CHUNKS):
                    t = smax.tile([uu, NL], FP32R, tag=f"cT{ui}")
                    coeft_sb.append(t)
                for ti, (n0, nn) in enumerate(N_CHUNKS):
                    for ui, (u0, uu) in enumerate(N_CHUNKS):
                        ps = psT.tile([uu, nn], FP32R, tag="psTc")
                        nc.tensor.transpose(
                            ps[:], coef_sb[ti][:, u0:u0 + uu],
                            identr[0:nn, 0:nn])
                        nc.vector.tensor_copy(
                            coeft_sb[ui][:, n0:n0 + nn], ps[:])

            # ---- stage F: out2 = coef @ h + gat_bias; stage G: out1 --
            with (
                tc.tile_pool(name="psF", bufs=2, space="PSUM") as psF,
                tc.tile_pool(name="psG", bufs=3, space="PSUM") as psG,
            ):
                for ti, (n0, nn) in enumerate(N_CHUNKS):
                    o2 = outp.tile([nn, D], FP32, tag=f"o2{ti}")
                    for half in range(2):
                        ps = psF.tile([nn, 300], FP32, tag="psF")
                        for ui in range(3):
                            mmr(ps[:], coeft_sb[ui][:, n0:n0 + nn],
                                h_sb[ui][:, half * 300:(half + 1) * 300],
                                start=(ui == 0), stop=(ui == 2))
                        nc.vector.tensor_add(
                            o2[:, half * 300:(half + 1) * 300], ps[:],
                            gb_bc[0:nn, half * 300:(half + 1) * 300])
                    nc.sync.dma_start(out2_d[n0:n0 + nn, :], o2[:])
                    # broadcast-write cause rows over S
                    if ti == 0:
                        rows = [(64, 64, 0)]
                    elif ti == 1:
                        rows = [(0, 128, 64)]
                    else:
                        rows = [(0, 64, 192)]
                    for (p0, cnt, c0) in rows:
                        step = 32
                        for off in range(0, cnt, step):
                            src = o2[p0 + off:p0 + off + step, :]
                            src = src.unsqueeze(1).broadcast_to(
                                [step, S, D])
                            nc.sync.dma_start(
                                outf_d[c0 + off:c0 + off + step, :, :], src)

                for ti, (n0, nn) in enumerate(N_CHUNKS):
                    o1 = outp.tile([nn, D], FP32, tag=f"o1{ti}")
                    for j, (j0, jj) in enumerate(D_SLABS):
                        ps = psG.tile([nn, 120], FP32, tag="psG")
                        nc.tensor.transpose(
                            ps[:], out1t_sb[j][:, n0:n0 + nn],
                            ident[0:120, 0:120])
                        nc.vector.tensor_copy(o1[:, j0:j0 + jj], ps[:])
                    nc.sync.dma_start(out1_d[n0:n0 + nn, :], o1[:])


def _host_prep(inputs):
    """Shard inputs across cores; build dense per-core graph operators."""
    tn = np.asarray(inputs["target_node"], np.float32)
    cn = np.asarray(inputs["cause_node"], np.float32)
    en = np.asarray(inputs["emotion_node"], np.float32)
    ei = np.asarray(inputs["edge_index"])
    et = np.asarray(inputs["edge_type"])
    basis = np.asarray(inputs["basis"], np.float32)
    comp = np.asarray(inputs["comp"], np.float32)
    root = np.asarray(inputs["root"], np.float32)
    rb = np.asarray(inputs["rgcn_bias"], np.float32)
    gw = np.asarray(inputs["gat_w"], np.float32)
    asrc = np.asarray(inputs["att_src"], np.float32)
    adst = np.asarray(inputs["att_dst"], np.float32)
    gb = np.asarray(inputs["gat_bias"], np.float32)

    # global node id -> (core, local index)  [x order: tgt, emo, cause]
    n = np.arange(2 * B + B * K)
    core = np.where(n < B, n // G,
                    np.where(n < 2 * B, (n - B) // G, (n - 2 * B) // (G * K)))
    local = np.where(n < B, n % G,
                     np.where(n < 2 * B, G + (n - B) % G,
                              2 * G + (n - 2 * B) % (G * K)))

    src, dst = ei[0], ei[1]
    ec = core[dst]
    sl = local[src]
    dl = local[dst]

    ct = np.zeros((NCORES, R, NL, NL), np.float32)   # [core, r, src, dst]
    np.add.at(ct, (ec, et, sl, dl), 1.0)
    cnt = ct.sum(axis=2)                             # [core, r, dst]
    ct /= np.maximum(cnt, 1.0)[:, :, None, :]
    # pad the src axis into 3 k-slabs of 128 (zero rows 64..127 of slab 2)
    ct_pad = np.zeros((NCORES, R, 3, 128, NL), np.float32)
    for kk, (n0, nn) in enumerate(((0, 128), (128, 128), (256, 64))):
        ct_pad[:, :, kk, 0:nn, :] = ct[:, :, n0:n0 + nn, :]
    ct_pad = ct_pad.reshape(NCORES, R * 3 * 128, NL)

    m = np.zeros((NCORES, NL, NL), np.float32)       # [core, dst, src]
    m[ec, dl, sl] = 1.0
    idx = np.arange(NL)
    m[:, idx, idx] = 1.0
    mb = np.where(m > 0, np.float32(0.0), np.float32(-1e30))

    wr = np.einsum("rb,bio->rio", comp, basis).astype(np.float32)
    wcat = np.concatenate([wr.reshape(R * D, D), root], axis=0)
    wcat = np.ascontiguousarray(wcat, np.float32)
    rb_slab = np.ascontiguousarray(rb.reshape(5, 120).T, np.float32)
    att = np.ascontiguousarray(np.stack([asrc, adst], axis=1), np.float32)
    gb_row = np.ascontiguousarray(gb[None, :], np.float32)
    gw = np.ascontiguousarray(gw, np.float32)

    ident_f = np.eye(128, dtype=np.float32)
    ones_row = np.ones((1, 128), np.float32)
    in_maps = []
    for c in range(NCORES):
        g0 = c * G
        x = np.concatenate([tn[g0:g0 + G], en[g0:g0 + G],
                            cn[g0 * K:(g0 + G) * K]], axis=0)
        x = np.ascontiguousarray(x, np.float32)
        xpad = np.zeros((3, 128, D), np.float32)
        for kk, (n0, nn) in enumerate(((0, 128), (128, 128), (256, 64))):
            xpad[kk, 0:nn, :] = x[n0:n0 + nn, :]
        in_maps.append({
            "x": xpad.reshape(3 * 128, D),
            "xt": np.ascontiguousarray(x.T),
            "idf": ident_f,
            "idr": ident_f,
            "ones": ones_row,
            "ct": np.ascontiguousarray(ct_pad[c]),
            "mb": np.ascontiguousarray(mb[c]),
            "wcat": wcat,
            "gw": gw,
            "att": att,
            "rb": rb_slab,
            "gb": gb_row,
        })
    return in_maps


def _run(inputs, trace=False, trace_kwargs=None):
    global _compiled
    if _compiled is None:
        _compiled = _build_program()
    nc = _compiled
    in_maps = _host_prep(inputs)
    kw = {}
    if trace:
        kw = dict(trace=True, trace_kwargs=trace_kwargs or {})
    res = run_bass_kernel_spmd(nc, in_maps, list(range(NCORES)), **kw)

    out1 = np.empty((2 * B + B * K, D), np.float32)
    out2 = np.empty_like(out1)
    outf = np.empty((B * K, S, D), np.float32)
    for c in range(NCORES):
        r = res.results[c]
        g0 = c * G
        o1, o2 = r["out1"], r["out2"]
        out1[g0:g0 + G] = o1[0:G]
        out1[B + g0:B + g0 + G] = o1[G:2 * G]
        out1[2 * B + g0 * K:2 * B + (g0 + G) * K] = o1[2 * G:]
        out2[g0:g0 + G] = o2[0:G]
        out2[B + g0:B + g0 + G] = o2[G:2 * G]
        out2[2 * B + g0 * K:2 * B + (g0 + G) * K] = o2[2 * G:]
        outf[g0 * K:(g0 + G) * K] = r["outf"]
    return (outf, out1, out2), res


def kernel(**inputs):
    (outf, out1, out2), _ = _run(inputs)
    return outf, out1, out2


# revision 28
# speedup vs baseline: 1.6782x; 1.6782x over previous
"""Trainium2 Bass kernel for nn_CausalGCN (RGCN + GAT message passing).

Sharding: data-parallel over graphs. Each of the 8 cores takes 32 of the
256 dialogs; the per-graph block-diagonal edge structure means no
cross-core edges. Small RGCN/GAT weights are replicated to every core.

Per-core math (N_loc = 320 nodes = 32 tgt + 32 emo + 256 cause, D = 600):
  RGCN   out1 = sum_r (Cn_r @ x) @ W_r + x @ root + b
         with Cn_r the row-normalized per-relation adjacency (dense
         [320,320], built on host from edge_index as part of sharding;
         mean-then-project == project-then-mean since W_r is linear).
  GAT    h = out1 @ gat_w; dense masked softmax over incoming edges
         (+self loops); out2 = coef @ h + b.
  Output out_final = broadcast of cause rows of out2 over S=128, written
         directly from SBUF with stride-0 source DMAs (memory-bound part).

All matmuls run fp32 on the PE (fp32 streams at the same N-cycles rate as
bf16 on trn2, so compute stays far below the HBM-write roofline).
"""

import sys

for _p in ("/opt/trn_rl_repo", "/root/.axon_site/_ro/trn_rl_repo"):
    if _p not in sys.path:
        sys.path.insert(0, _p)

import numpy as np

import concourse.bacc as bacc
import concourse.mybir as mybir
import concourse.tile as tile
from concourse.bass_utils import run_bass_kernel_spmd

# Problem constants (hardcoded per the task contract).
B = 256       # graphs (dialogs)
K = 8         # cause utterances per graph
S = 128       # sequence length of the broadcast output
D = 600       # node/hidden dim
R = 8         # relations
NCORES = 8
G = B // NCORES          # 32 graphs per core
NL = G * (K + 2)         # 320 local nodes per core
NC_CAUSE = G * K         # 256 local cause nodes per core
KDIM = R * D + D         # 5400 contraction dim for [Wr..., root]

FP32 = mybir.dt.float32
FP32R = mybir.dt.float32r  # fp32 bits, 4x faster PE streaming at N>=256

# Node-chunk tiling along the 320-node axis (PE partition limit 128).
N_CHUNKS = [(0, 128), (128, 128), (256, 64)]
# Feature-dim slabs along D=600 (5 x 120).
D_SLABS = [(j * 120, 120) for j in range(5)]

_compiled = None  # (nc, ) cache — compile once per process


def _build_program(repeat=1, no_outf=False):
    nc = bacc.Bacc("TRN2", target_bir_lowering=False, debug=False,
                   num_devices=NCORES)

    x_d = nc.dram_tensor("x", [3 * 128, D], FP32R, kind="ExternalInput")
    xt_d = nc.dram_tensor("xt", [D, NL], FP32R, kind="ExternalInput")
    # ct padded to k-slabs of 128 (zero rows) so it loads as one DMA
    ct_d = nc.dram_tensor("ct", [R * 3 * 128, NL], FP32R,
                          kind="ExternalInput")
    mb_d = nc.dram_tensor("mb", [NL, NL], FP32, kind="ExternalInput")
    wcat_d = nc.dram_tensor("wcat", [KDIM, D], FP32R, kind="ExternalInput")
    gw_d = nc.dram_tensor("gw", [D, D], FP32R, kind="ExternalInput")
    att_d = nc.dram_tensor("att", [D, 2], FP32R, kind="ExternalInput")
    rb_d = nc.dram_tensor("rb", [120, 5], FP32, kind="ExternalInput")
    id_d = nc.dram_tensor("idf", [128, 128], FP32, kind="ExternalInput")
    idr_d = nc.dram_tensor("idr", [128, 128], FP32R, kind="ExternalInput")
    ones_d = nc.dram_tensor("ones", [1, 128], FP32R, kind="ExternalInput")
    gb_d = nc.dram_tensor("gb", [1, D], FP32, kind="ExternalInput")

    out1_d = nc.dram_tensor("out1", [NL, D], FP32, kind="ExternalOutput")
    out2_d = nc.dram_tensor("out2", [NL, D], FP32, kind="ExternalOutput")
    outf_d = nc.dram_tensor("outf", [NC_CAUSE, S, D], FP32,
                            kind="ExternalOutput")

    dd = dict(x_d=x_d, xt_d=xt_d, ct_d=ct_d, mb_d=mb_d, wcat_d=wcat_d,
              gw_d=gw_d, att_d=att_d, rb_d=rb_d, id_d=id_d, idr_d=idr_d,
              ones_d=ones_d, gb_d=gb_d, out1_d=out1_d, out2_d=out2_d,
              outf_d=outf_d)
    with tile.TileContext(nc) as tc:
        if repeat == 1:
            _emit_body(nc, tc, dd, no_outf)
        else:
            with tc.For_i(0, repeat, 1):
                _emit_body(nc, tc, dd, no_outf)

    nc.compile()
    return nc


def _emit_body(nc, tc, dd, no_outf=False):
    (x_d, xt_d, ct_d, mb_d, wcat_d, gw_d, att_d, rb_d, id_d, idr_d,
     ones_d, gb_d, out1_d, out2_d, outf_d) = (
        dd[k] for k in ("x_d", "xt_d", "ct_d", "mb_d", "wcat_d", "gw_d",
                        "att_d", "rb_d", "id_d", "idr_d", "ones_d",
                        "gb_d", "out1_d", "out2_d", "outf_d"))

    def mmr(out, lhsT, rhs, start, stop):
        nc.tensor.matmul(out, lhsT, rhs, start=start, stop=stop)

    if True:
        with (
            tc.tile_pool(name="const", bufs=1) as const,
            tc.tile_pool(name="ctp", bufs=1) as ctp,
            tc.tile_pool(name="aggp", bufs=1) as aggp,
            tc.tile_pool(name="wstream", bufs=2) as wstream,
            tc.tile_pool(name="o1t", bufs=1) as o1tp,
            tc.tile_pool(name="hp", bufs=1) as hp,
            tc.tile_pool(name="htp", bufs=1) as htp,
            tc.tile_pool(name="smax", bufs=1) as smax,
            tc.tile_pool(name="outp", bufs=1) as outp,
        ):
            # ---- resident inputs -------------------------------------
            # x loaded into zero-padded k-slab layout [128, 3, D]
            xpad = ctp.tile([128, 3, D], FP32R, tag="xpad")
            nc.sync.dma_start(
                xpad[:], x_d[:].rearrange("(k p) d -> p k d", k=3))
            xt_sb = []
            for (j0, jj) in D_SLABS:
                t = const.tile([jj, NL], FP32R, tag=f"xt{j0}")
                nc.sync.dma_start(t[:], xt_d[j0:j0 + jj, :])
                xt_sb.append(t)
            gw_all = const.tile([120, 5, D], FP32R, tag="gw")
            nc.sync.dma_start(
                gw_all[:], gw_d[:].rearrange("(s p) d -> p s d", s=5))
            gw_sb = [gw_all[:, j, :] for j in range(5)]
            att_sb = []
            for (j0, jj) in D_SLABS:
                t = const.tile([jj, 2], FP32R, tag=f"att{j0}")
                nc.sync.dma_start(t[:], att_d[j0:j0 + jj, :])
                att_sb.append(t)
            rb_sb = const.tile([120, 5], FP32, tag="rb")
            nc.sync.dma_start(rb_sb[:], rb_d[:])
            gb_bc = const.tile([128, D], FP32, tag="gb")
            nc.sync.dma_start(gb_bc[:], gb_d[0:1, :].broadcast_to([128, D]))
            mb_sb = []
            for (n0, nn) in N_CHUNKS:
                t = const.tile([nn, NL], FP32, tag=f"mb{n0}")
                nc.sync.dma_start(t[:], mb_d[n0:n0 + nn, :])
                mb_sb.append(t)
            ident = const.tile([128, 128], FP32, tag="ident")
            identr = const.tile([128, 128], FP32R, tag="identr")
            nc.sync.dma_start(ident[:], id_d[:])
            nc.sync.dma_start(identr[:], idr_d[:])

            # ---- stage A: aggT_r[di, n] = sum_s x[s, di] * Cn_r[s, n] --
            # ct_sb[:, r*3+kk, :] is the (r, k-slab) rhs; last slab is
            # zero-padded from 64 to 128 rows so it loads as one DMA.
            agg_sb = []
            ct_half = None
            with tc.tile_pool(name="psA", bufs=2, space="PSUM") as psA:
                for r in range(R):
                    if r % 4 == 0:
                        # stream ct in two half-tensors (4 relations each)
                        # to halve its SBUF footprint
                        ct_half = ctp.tile([128, 12, NL], FP32R, tag="ct",
                                           name=f"cth{r // 4}")
                        h0 = (r // 4) * 12 * 128
                        nc.sync.dma_start(
                            ct_half[:],
                            ct_d[h0:h0 + 12 * 128, :].rearrange(
                                "(rk p) d -> p rk d", rk=12))
                    agg_r = aggp.tile([120, 5, NL], FP32R, tag=f"agg{r}")
                    for mj, (m0, mw) in enumerate(D_SLABS):
                        ps = psA.tile([120, NL], FP32, tag="psA")
                        for kk in range(3):
                            mmr(ps[:], xpad[:, kk, m0:m0 + mw],
                                ct_half[:, (r % 4) * 3 + kk, :],
                                start=(kk == 0), stop=(kk == 2))
                        nc.vector.tensor_copy(agg_r[:, mj, :], ps[:])
                    agg_sb.append(agg_r)

                # ---- stage B: out1T = wcat.T-contract, k-streamed ------
                with tc.tile_pool(name="psB", bufs=1, space="PSUM") as psB:
                    psb = [psB.tile([120, NL], FP32, tag=f"psB{m}",
                                    name=f"psb{m}")
                           for m in range(5)]
                    nk = R * 5 + 5
                    for k9 in range(9):  # 9 super-slabs of 5 k-slabs each
                        wt = wstream.tile([120, 5, D], FP32R, tag="w")
                        nc.sync.dma_start(
                            wt[:],
                            wcat_d[k9 * 600:(k9 + 1) * 600, :].rearrange(
                                "(s p) d -> p s d", s=5))
                        for s in range(5):
                            k = k9 * 5 + s
                            if k < R * 5:
                                rhs = agg_sb[k // 5][:, k % 5, :]
                            else:
                                rhs = xt_sb[k - R * 5][:]
                            for mj, (m0, mw) in enumerate(D_SLABS):
                                mmr(psb[mj][:], wt[:, s, m0:m0 + mw], rhs,
                                    start=(k == 0), stop=(k == nk - 1))
                    out1t_sb = []
                    out1tr_sb = []
                    for mj in range(5):
                        t = o1tp.tile([120, NL], FP32, tag=f"o1t{mj}")
                        nc.vector.tensor_scalar_add(
                            t[:], psb[mj][:], rb_sb[:, mj:mj + 1])
                        out1t_sb.append(t)
                        tr = o1tp.tile([120, NL], FP32R, tag=f"o1tr{mj}",
                                       name=f"o1tr{mj}")
                        nc.vector.tensor_scalar_add(
                            tr[:], psb[mj][:], rb_sb[:, mj:mj + 1])
                        out1tr_sb.append(tr)

            # ---- stage C: h (native), hT, attention alphas -----------
            h_sb = []
            with (
                tc.tile_pool(name="psC", bufs=2, space="PSUM") as psC,
                tc.tile_pool(name="psD", bufs=2, space="PSUM") as psD,
                tc.tile_pool(name="psE", bufs=1, space="PSUM") as psE,
            ):
                for ti, (n0, nn) in enumerate(N_CHUNKS):
                    t = hp.tile([nn, D], FP32R, tag=f"h{ti}")
                    for half in range(2):
                        ps = psC.tile([nn, 300], FP32, tag="psC")
                        for j in range(5):
                            mmr(ps[:], out1tr_sb[j][:, n0:n0 + nn],
                                gw_sb[j][:, half * 300:(half + 1) * 300],
                                start=(j == 0), stop=(j == 4))
                        nc.vector.tensor_copy(
                            t[:, half * 300:(half + 1) * 300], ps[:])
                    h_sb.append(t)
                ht_sb = []
                for j2, (o0, oo) in enumerate(D_SLABS):
                    t = htp.tile([oo, NL], FP32R, tag=f"ht{j2}")
                    ps = psD.tile([oo, NL], FP32, tag="psD")
                    for j in range(5):
                        mmr(ps[:], gw_sb[j][:, o0:o0 + oo], out1tr_sb[j][:],
                            start=(j == 0), stop=(j == 4))
                    nc.vector.tensor_copy(t[:], ps[:])
                    ht_sb.append(t)
                psal = psE.tile([2, NL], FP32, tag="psE")
                for j in range(5):
                    mmr(psal[:], att_sb[j][:], ht_sb[j][:],
                        start=(j == 0), stop=(j == 4))
                alphas = smax.tile([2, NL], FP32R, tag="alphas")
                nc.vector.tensor_copy(alphas[:], psal[:])

                # a_src broadcast row: ones[1,128].T @ alphas[0:1,:]
                ones_row = smax.tile([1, 128], FP32R, tag="ones")
                nc.sync.dma_start(ones_row[:], ones_d[:])
                asrc_bc = smax.tile([128, NL], FP32, tag="asrc")
                psbc = psE.tile([128, NL], FP32, tag="psBc")
                mmr(psbc[:], ones_row[:], alphas[0:1, :],
                    start=True, stop=True)
                nc.vector.tensor_copy(asrc_bc[:], psbc[:])
            acol_sb = []
            with tc.tile_pool(name="psT", bufs=2, space="PSUM") as psT:
                for ti, (n0, nn) in enumerate(N_CHUNKS):
                    ps = psT.tile([nn, 2], FP32R, tag="psTa")
                    nc.tensor.transpose(ps[:], alphas[:, n0:n0 + nn],
                                        identr[0:2, 0:2])
                    t = smax.tile([nn, 2], FP32, tag=f"acol{ti}")
                    nc.vector.tensor_copy(t[:], ps[:])
                    acol_sb.append(t)

                # ---- stage D: masked softmax over incoming edges ------
                coef_sb = []
                for ti, (n0, nn) in enumerate(N_CHUNKS):
                    L = smax.tile([nn, NL], FP32, tag=f"L{ti}")
                    nc.vector.tensor_scalar_add(
                        L[:], asrc_bc[0:nn, :], acol_sb[ti][:, 1:2])
                    # leaky_relu(x, 0.2) = max(x, 0.2x)
                    Lm = smax.tile([nn, NL], FP32, tag="Lm")
                    nc.vector.tensor_scalar_mul(Lm[:], L[:], 0.2)
                    nc.vector.tensor_max(L[:], L[:], Lm[:])
                    nc.vector.tensor_add(L[:], L[:], mb_sb[ti][:])
                    namax = smax.tile([nn, 1], FP32, tag="namax")
                    nc.vector.tensor_reduce(
                        namax[:], L[:], axis=mybir.AxisListType.X,
                        op=mybir.AluOpType.max, negate=True)
                    P = smax.tile([nn, NL], FP32R, tag=f"P{ti}")
                    den = smax.tile([nn, 1], FP32, tag="den")
                    nc.scalar.activation(
                        P[:], L[:], mybir.ActivationFunctionType.Exp,
                        bias=namax[:], accum_out=den[:])
                    rden = smax.tile([nn, 1], FP32, tag="rden")
                    nc.vector.reciprocal(rden[:], den[:])
                    nc.vector.tensor_scalar_mul(P[:], P[:], rden[:])
                    coef_sb.append(P)

                # ---- stage E: coefT via PE transpose ------------------
                coeft_sb = []
                for ui, (u0, uu) in enumerate(N_CHUNKS):
                    t = smax.tile([uu, NL], FP32R, tag=f"cT{ui}")
                    coeft_sb.append(t)
                for ti, (n0, nn) in enumerate(N_CHUNKS):
                    for ui, (u0, uu) in enumerate(N_CHUNKS):
                        ps = psT.tile([uu, nn], FP32R, tag="psTc")
                        nc.tensor.transpose(
                            ps[:], coef_sb[ti][:, u0:u0 + uu],
                            identr[0:nn, 0:nn])
                        nc.vector.tensor_copy(
                            coeft_sb[ui][:, n0:n0 + nn], ps[:])

            # ---- stage F: out2 = coef @ h + gat_bias; stage G: out1 --
            with (
                tc.tile_pool(name="psF", bufs=2, space="PSUM") as psF,
                tc.tile_pool(name="psG", bufs=3, space="PSUM") as psG,
            ):
                for ti, (n0, nn) in enumerate(N_CHUNKS):
                    o2 = outp.tile([nn, D], FP32, tag=f"o2{ti}")
                    for half in range(2):
                        ps = psF.tile([nn, 300], FP32, tag="psF")
                        for ui in range(3):
                            mmr(ps[:], coeft_sb[ui][:, n0:n0 + nn],
                                h_sb[ui][:, half * 300:(half + 1) * 300],
                                start=(ui == 0), stop=(ui == 2))
                        nc.vector.tensor_add(
                            o2[:, half * 300:(half + 1) * 300], ps[:],
                            gb_bc[0:nn, half * 300:(half + 1) * 300])
                    nc.sync.dma_start(out2_d[n0:n0 + nn, :], o2[:])
                    # broadcast-write cause rows over S
                    if ti == 0:
                        rows = [(64, 64, 0)]
                    elif ti == 1:
                        rows = [(0, 128, 64)]
                    else:
                        rows = [(0, 64, 192)]
                    # A 4x replica of each cause row is materialized in
                    # SBUF first so the stride-0 DMA descriptors are
                    # 9.6KB instead of 2.4KB (the small-descriptor path
                    # only reaches ~half of HBM write bandwidth).
                    REP = 4
                    for (p0, cnt, c0) in (() if no_outf else rows):
                        rep_t = outp.tile([cnt, REP, D], FP32,
                                          tag=f"rep{ti}", name=f"rep{ti}")
                        for j in range(REP):
                            nc.vector.tensor_copy(
                                rep_t[:, j, :], o2[p0:p0 + cnt, :])
                        step = 32
                        for off in range(0, cnt, step):
                            src = rep_t[off:off + step, :, :]
                            src = src.unsqueeze(1).broadcast_to(
                                [step, S // REP, REP, D])
                            dst = outf_d[c0 + off:c0 + off + step, :, :]
                            dst = dst.rearrange("c (g x) d -> c g x d",
                                                x=REP)
                            nc.sync.dma_start(dst, src)

                for ti, (n0, nn) in enumerate(N_CHUNKS):
                    o1 = outp.tile([nn, D], FP32, tag=f"o1{ti}")
                    for j, (j0, jj) in enumerate(D_SLABS):
                        ps = psG.tile([nn, 120], FP32, tag="psG")
                        nc.tensor.transpose(
                            ps[:], out1t_sb[j][:, n0:n0 + nn],
                            ident[0:120, 0:120])
                        nc.vector.tensor_copy(o1[:, j0:j0 + jj], ps[:])
                    nc.sync.dma_start(out1_d[n0:n0 + nn, :], o1[:])


def _host_prep(inputs):
    """Shard inputs across cores; build dense per-core graph operators."""
    tn = np.asarray(inputs["target_node"], np.float32)
    cn = np.asarray(inputs["cause_node"], np.float32)
    en = np.asarray(inputs["emotion_node"], np.float32)
    ei = np.asarray(inputs["edge_index"])
    et = np.asarray(inputs["edge_type"])
    basis = np.asarray(inputs["basis"], np.float32)
    comp = np.asarray(inputs["comp"], np.float32)
    root = np.asarray(inputs["root"], np.float32)
    rb = np.asarray(inputs["rgcn_bias"], np.float32)
    gw = np.asarray(inputs["gat_w"], np.float32)
    asrc = np.asarray(inputs["att_src"], np.float32)
    adst = np.asarray(inputs["att_dst"], np.float32)
    gb = np.asarray(inputs["gat_bias"], np.float32)

    # global node id -> (core, local index)  [x order: tgt, emo, cause]
    n = np.arange(2 * B + B * K)
    core = np.where(n < B, n // G,
                    np.where(n < 2 * B, (n - B) // G, (n - 2 * B) // (G * K)))
    local = np.where(n < B, n % G,
                     np.where(n < 2 * B, G + (n - B) % G,
                              2 * G + (n - 2 * B) % (G * K)))

    src, dst = ei[0], ei[1]
    ec = core[dst]
    sl = local[src]
    dl = local[dst]

    ct = np.zeros((NCORES, R, NL, NL), np.float32)   # [core, r, src, dst]
    np.add.at(ct, (ec, et, sl, dl), 1.0)
    cnt = ct.sum(axis=2)                             # [core, r, dst]
    ct /= np.maximum(cnt, 1.0)[:, :, None, :]
    # pad the src axis into 3 k-slabs of 128 (zero rows 64..127 of slab 2)
    ct_pad = np.zeros((NCORES, R, 3, 128, NL), np.float32)
    for kk, (n0, nn) in enumerate(((0, 128), (128, 128), (256, 64))):
        ct_pad[:, :, kk, 0:nn, :] = ct[:, :, n0:n0 + nn, :]
    ct_pad = ct_pad.reshape(NCORES, R * 3 * 128, NL)

    m = np.zeros((NCORES, NL, NL), np.float32)       # [core, dst, src]
    m[ec, dl, sl] = 1.0
    idx = np.arange(NL)
    m[:, idx, idx] = 1.0
    mb = np.where(m > 0, np.float32(0.0), np.float32(-1e30))

    wr = np.einsum("rb,bio->rio", comp, basis).astype(np.float32)
    wcat = np.concatenate([wr.reshape(R * D, D), root], axis=0)
    wcat = np.ascontiguousarray(wcat, np.float32)
    rb_slab = np.ascontiguousarray(rb.reshape(5, 120).T, np.float32)
    att = np.ascontiguousarray(np.stack([asrc, adst], axis=1), np.float32)
    gb_row = np.ascontiguousarray(gb[None, :], np.float32)
    gw = np.ascontiguousarray(gw, np.float32)

    ident_f = np.eye(128, dtype=np.float32)
    ones_row = np.ones((1, 128), np.float32)
    in_maps = []
    for c in range(NCORES):
        g0 = c * G
        x = np.concatenate([tn[g0:g0 + G], en[g0:g0 + G],
                            cn[g0 * K:(g0 + G) * K]], axis=0)
        x = np.ascontiguousarray(x, np.float32)
        xpad = np.zeros((3, 128, D), np.float32)
        for kk, (n0, nn) in enumerate(((0, 128), (128, 128), (256, 64))):
            xpad[kk, 0:nn, :] = x[n0:n0 + nn, :]
        in_maps.append({
            "x": xpad.reshape(3 * 128, D),
            "xt": np.ascontiguousarray(x.T),
            "idf": ident_f,
            "idr": ident_f,
            "ones": ones_row,
            "ct": np.ascontiguousarray(ct_pad[c]),
            "mb": np.ascontiguousarray(mb[c]),
            "wcat": wcat,
            "gw": gw,
            "att": att,
            "rb": rb_slab,
            "gb": gb_row,
        })
    return in_maps


def _run(inputs, trace=False, trace_kwargs=None):
    global _compiled
    if _compiled is None:
        _compiled = _build_program()
    nc = _compiled
    in_maps = _host_prep(inputs)
    kw = {}
    if trace:
        kw = dict(trace=True, trace_kwargs=trace_kwargs or {})
    res = run_bass_kernel_spmd(nc, in_maps, list(range(NCORES)), **kw)

    out1 = np.empty((2 * B + B * K, D), np.float32)
    out2 = np.empty_like(out1)
    outf = np.empty((B * K, S, D), np.float32)
    for c in range(NCORES):
        r = res.results[c]
        g0 = c * G
        o1, o2 = r["out1"], r["out2"]
        out1[g0:g0 + G] = o1[0:G]
        out1[B + g0:B + g0 + G] = o1[G:2 * G]
        out1[2 * B + g0 * K:2 * B + (g0 + G) * K] = o1[2 * G:]
        out2[g0:g0 + G] = o2[0:G]
        out2[B + g0:B + g0 + G] = o2[G:2 * G]
        out2[2 * B + g0 * K:2 * B + (g0 + G) * K] = o2[2 * G:]
        outf[g0 * K:(g0 + G) * K] = r["outf"]
    return (outf, out1, out2), res


def kernel(**inputs):
    (outf, out1, out2), _ = _run(inputs)
    return outf, out1, out2
